# revision 1
# baseline (speedup 1.0000x reference)
"""Trainium2 Bass kernel for nn_ConditionalFeedForward (MoE top-2 routing).

Strategy: expert-parallel across the 8 NeuronCores — core e owns expert e's
weights. Host (numpy) gathers each expert's routed tokens (multi-hot
routing_map), pads to a common capacity CAP, and pre-transposes/pre-tiles
operands into PE-friendly layouts. Each core computes, for its expert:

    hT = silu(w1 @ xT) * (w3 @ xT)          # [FFN, CAP] staged via DRAM
    yT = w2 @ hT                            # [DIM, CAP]

with float32r matmuls (full PE rate, ~1e-4 relative error), SwiGLU fused on
ScalarE (Silu) + VectorE (mul). Host scatter-adds gate-weighted outputs back
to the full [N_TOKENS, DIM] result.
"""

import os
import numpy as np

import concourse.bacc as bacc
import concourse.mybir as mybir
import concourse.tile as tile
from concourse.bass_utils import run_bass_kernel_spmd

# Problem constants (hardcoded per harness contract)
NUM_EXPERTS = 8
DIM = 2048
FFN = 5632
N_CORES = 8
KD = DIM // 128    # 16 contraction tiles for GEMM1/3, output tiles for GEMM2
KF = FFN // 128    # 44 ffn chunks

F32 = mybir.dt.float32
F32R = mybir.dt.float32r

# Compiled program cache keyed by CAP
_PROGRAMS = {}

# Filled by the last kernel() call when BASS_KERNEL_TRACE=1 (for test.py)
LAST_EXEC_NS = None


def _split(total, hi, lo=256):
    """Split `total` into (offset, size) parts, each size in [lo, hi],
    preferring `hi`-sized parts. Requires total >= lo."""
    assert total >= lo
    parts = []
    rem = total
    while rem > hi:
        take = hi if rem - hi >= lo else rem - lo
        parts.append(take)
        rem -= take
    parts.append(rem)
    out = []
    t0 = 0
    for p in parts:
        out.append((t0, p))
        t0 += p
    return out


def _p1_tiles(cap):
    """Phase-1 token tiles: matmul N in [256,512] for full f32r rate."""
    return _split(cap, 512)


def _p2_blocks(cap):
    """Phase-2 token blocks of <=768 (SBUF-resident hT slab per block).
    Largest first: only the smaller later blocks' hT reloads are exposed
    at block boundaries (the slab slot serializes load-after-compute)."""
    return _split(cap, 768)


def _p2_subtiles(bn):
    """Split a block into PSUM-bank-sized matmul N-tiles (each in [256,512])."""
    return _split(bn, 512)


def _build_program(cap):
    nc = bacc.Bacc("TRN2", target_bir_lowering=False, debug=False,
                   num_devices=N_CORES)

    xt_d = nc.dram_tensor("xt", [KD, 128, cap], F32R, kind="ExternalInput")
    w1l_d = nc.dram_tensor("w1l", [KF, 128, KD, 128], F32R, kind="ExternalInput")
    w3l_d = nc.dram_tensor("w3l", [KF, 128, KD, 128], F32R, kind="ExternalInput")
    w2l_d = nc.dram_tensor("w2l", [KD, 128, KF, 128], F32R, kind="ExternalInput")
    yt_d = nc.dram_tensor("yt", [KD, 128, cap], F32, kind="ExternalOutput")
    htb_d = nc.dram_tensor("htb", [KF, 128, cap], F32R, kind="Internal")

    silu = mybir.ActivationFunctionType.Silu
    p1t = _p1_tiles(cap)

    with tile.TileContext(nc) as tc:
        # ---- Phase 1: hT = silu(w1 @ xT) * (w3 @ xT), staged to DRAM ----
        with (
            tc.tile_pool(name="xt", bufs=1) as xpool,
            tc.tile_pool(name="w13", bufs=2) as wpool,
            tc.tile_pool(name="hst", bufs=3) as spool,
            tc.tile_pool(name="ps1", bufs=3, space="PSUM") as psum1,
        ):
            # x loads on the ACT HWDGE ring (idle at kernel start; the SP ring
            # carries the weight stream). Sliced per token-tile so the first
            # PSUM group's 16 k-chunks arrive ASAP.
            # x loads on SWDGE (gpsimd): the SP ring carries the weight
            # stream and the ACT ring carries compute-result stores.
            xt_s = xpool.tile([128, KD, cap], F32R)
            for k in range(KD):
                nc.gpsimd.dma_start(xt_s[:, k, :], xt_d[k])
            for f in range(KF):
                w1c = wpool.tile([128, KD, 128], F32R, tag="w1c")
                nc.sync.dma_start(w1c[:], w1l_d[f])
                w3c = wpool.tile([128, KD, 128], F32R, tag="w3c")
                nc.sync.dma_start(w3c[:], w3l_d[f])
                for (t0, tn) in p1t:
                    h1p = psum1.tile([128, tn], F32, tag="h1p")
                    h3p = psum1.tile([128, tn], F32, tag="h3p")
                    for k in range(KD):
                        nc.tensor.matmul(
                            h1p[:], w1c[:, k, :], xt_s[:, k, t0:t0 + tn],
                            start=(k == 0), stop=(k == KD - 1))
                    for k in range(KD):
                        nc.tensor.matmul(
                            h3p[:], w3c[:, k, :], xt_s[:, k, t0:t0 + tn],
                            start=(k == 0), stop=(k == KD - 1))
                    s1 = spool.tile([128, tn], F32, tag="s1")
                    nc.scalar.activation(s1[:], h1p[:], silu)
                    ht = spool.tile([128, tn], F32, tag="ht")
                    nc.vector.tensor_mul(ht[:], s1[:], h3p[:])
                    nc.scalar.dma_start(htb_d[f][:, t0:t0 + tn],
                                        ht[:].bitcast(F32R))

        # ---- Phase 2: yT = w2 @ hT ----
        with (
            tc.tile_pool(name="htk", bufs=1) as hpool,
            tc.tile_pool(name="w2", bufs=2) as w2pool,
            tc.tile_pool(name="yo", bufs=3) as ypool,
            tc.tile_pool(name="ps2", bufs=4, space="PSUM") as psum2,
        ):
            for (b0, bn) in _p2_blocks(cap):
                # hT reloads on SWDGE: they must not sit behind the weight
                # stream in the SP HWDGE FIFO (they become ready much earlier)
                htk = hpool.tile([128, KF, bn], F32R, tag="htk")
                for k2 in range(KF):
                    nc.gpsimd.dma_start(htk[:, k2, :], htb_d[k2][:, b0:b0 + bn])
                for m in range(KD):
                    # chunk-split so the first matmuls of this m start after
                    # ~1/4 of the weight load instead of all of it
                    w2c = w2pool.tile([128, KF, 128], F32R, tag="w2c")
                    for c in range(4):
                        nc.sync.dma_start(w2c[:, c * 11:(c + 1) * 11, :],
                                          w2l_d[m][:, c * 11:(c + 1) * 11, :])
                    for (s0, sn) in _p2_subtiles(bn):
                        yp = psum2.tile([128, sn], F32, tag="yp")
                        for k2 in range(KF):
                            nc.tensor.matmul(
                                yp[:], w2c[:, k2, :], htk[:, k2, s0:s0 + sn],
                                start=(k2 == 0), stop=(k2 == KF - 1))
                        yo = ypool.tile([128, sn], F32, tag="yo")
                        nc.vector.tensor_copy(yo[:], yp[:])
                        nc.scalar.dma_start(
                            yt_d[m][:, b0 + s0:b0 + s0 + sn], yo[:])

    nc.compile()
    return nc


def kernel(x, expert_indices, expert_weights, w1, w2, w3):
    global LAST_EXEC_NS
    x = np.ascontiguousarray(np.asarray(x, dtype=np.float32))
    routing = np.asarray(expert_indices)
    probs = np.asarray(expert_weights, dtype=np.float32)
    w1 = np.asarray(w1, dtype=np.float32)
    w2 = np.asarray(w2, dtype=np.float32)
    w3 = np.asarray(w3, dtype=np.float32)
    n_tokens = x.shape[0]

    idxs = [np.flatnonzero(routing[:, e]) for e in range(NUM_EXPERTS)]
    max_count = max(len(i) for i in idxs)
    cap = max(512, -(-max_count // 16) * 16)  # round up to multiple of 16
    assert cap <= 2304, f"unexpectedly imbalanced routing: max_count={max_count}"

    if cap not in _PROGRAMS:
        _PROGRAMS[cap] = _build_program(cap)
    nc = _PROGRAMS[cap]

    def _prep(e):
        idx = idxs[e]
        xt = np.zeros((DIM, cap), dtype=np.float32)
        xt[:, :len(idx)] = x[idx].T
        return {
            "xt": xt.reshape(KD, 128, cap),
            # W1L[f,p,k,m] = w1[e][f*128+m, k*128+p]
            "w1l": np.ascontiguousarray(
                w1[e].reshape(KF, 128, KD, 128).transpose(0, 3, 2, 1)),
            "w3l": np.ascontiguousarray(
                w3[e].reshape(KF, 128, KD, 128).transpose(0, 3, 2, 1)),
            # W2L[m,p,k2,d] = w2[e][m*128+d, k2*128+p]
            "w2l": np.ascontiguousarray(
                w2[e].reshape(KD, 128, KF, 128).transpose(0, 3, 2, 1)),
        }

    from concurrent.futures import ThreadPoolExecutor
    with ThreadPoolExecutor(max_workers=NUM_EXPERTS) as pool:
        in_maps = list(pool.map(_prep, range(NUM_EXPERTS)))

    trace = os.environ.get("BASS_KERNEL_TRACE", "0") == "1"
    if trace:
        import importlib.util
        if importlib.util.find_spec("antenv") is None or importlib.util.find_spec(
                "antenv.axon_hooks") is None:
            trace = False  # NTFF hook unavailable in this environment
    res = run_bass_kernel_spmd(
        nc, in_maps, core_ids=list(range(N_CORES)),
        trace=trace, trace_cores=list(range(N_CORES)) if trace else None,
    )
    LAST_EXEC_NS = res.exec_time_ns

    out = np.zeros((n_tokens, DIM), dtype=np.float32)
    for e in range(NUM_EXPERTS):
        idx = idxs[e]
        y_t = res.results[e]["yt"].reshape(DIM, cap)[:, :len(idx)]
        out[idx] += probs[idx, e][:, None] * y_t.T
    return out



# revision 2
# speedup vs baseline: 1.1630x; 1.1630x over previous
"""Trainium2 Bass kernel for nn_ConditionalFeedForward (MoE top-2 routing).

Strategy: expert-parallel across 8 NeuronCores with a load-balancing "side
block". Core e owns expert e's weights and computes the first CAP_M routed
tokens of expert e; the overflow tokens of heavy experts are distributed as
<=CAP_S-token side blocks to other cores (each core carries one side block
with its own small weight stream). All matmul operands are fp16 (full
1 cycle/row PE rate, half the HBM traffic of fp32r); PSUM accumulates fp32.

Single fused pass per core, hT kept resident in SBUF (no DRAM staging):

    hT = silu(w1 @ xT) * (w3 @ xT)     # [FFN, CAP] fp16 slab in SBUF
    yT = w2 @ hT                       # [DIM, CAP] -> fp16 out

Host gathers/pads tokens per expert, pre-transposes weights into PE layouts,
and scatter-adds gate-weighted outputs back to the full [N_TOKENS, DIM]
result in fp32.
"""

import os
import numpy as np

import concourse.bacc as bacc
import concourse.mybir as mybir
import concourse.tile as tile
from concourse.bass_utils import run_bass_kernel_spmd

# Problem constants (hardcoded per harness contract)
NUM_EXPERTS = 8
DIM = 2048
FFN = 5632
N_CORES = 8
KD = DIM // 128    # 16 contraction chunks for GEMM1/3; output chunks GEMM2
KF = FFN // 128    # 44 ffn chunks

F32 = mybir.dt.float32
F16 = mybir.dt.float16

# Compiled program cache keyed by (cap_m, cap_s)
_PROGRAMS = {}

# Filled by the last kernel() call when BASS_KERNEL_TRACE=1 (for test.py)
LAST_EXEC_NS = None


def _tiles(total, mx=512):
    """Token tiles of <=512 (PSUM bank = 512 fp32)."""
    return [(t0, min(mx, total - t0)) for t0 in range(0, total, mx)]


def _build_program(cap_m, cap_s):
    nc = bacc.Bacc("TRN2", target_bir_lowering=False, debug=False,
                   num_devices=N_CORES)

    xm_d = nc.dram_tensor("xm", [128, KD, cap_m], F16, kind="ExternalInput")
    w1m_d = nc.dram_tensor("w1m", [KF, 128, KD, 128], F16, kind="ExternalInput")
    w3m_d = nc.dram_tensor("w3m", [KF, 128, KD, 128], F16, kind="ExternalInput")
    w2m_d = nc.dram_tensor("w2m", [KD, 128, KF, 128], F16, kind="ExternalInput")
    ym_d = nc.dram_tensor("ym", [KD, 128, cap_m], F16, kind="ExternalOutput")
    if cap_s:
        xs_d = nc.dram_tensor("xs", [128, KD, cap_s], F16, kind="ExternalInput")
        w1s_d = nc.dram_tensor("w1s", [KF, 128, KD, 128], F16, kind="ExternalInput")
        w3s_d = nc.dram_tensor("w3s", [KF, 128, KD, 128], F16, kind="ExternalInput")
        w2s_d = nc.dram_tensor("w2s", [KD, 128, KF, 128], F16, kind="ExternalInput")
        ys_d = nc.dram_tensor("ys", [KD, 128, cap_s], F16, kind="ExternalOutput")

    silu = mybir.ActivationFunctionType.Silu
    tiles_m = _tiles(cap_m)

    with tile.TileContext(nc) as tc:
        with (
            tc.tile_pool(name="x", bufs=1) as xpool,
            tc.tile_pool(name="h", bufs=1) as hpool,
        ):
            xm_s = xpool.tile([128, KD, cap_m], F16)
            hm_s = hpool.tile([128, KF, cap_m], F16)
            if cap_s:
                xs_s = xpool.tile([128, KD, cap_s], F16)
                hs_s = hpool.tile([128, KF, cap_s], F16)
            # x loads on the ACT HWDGE ring (single big partition-major DMAs;
            # the SP ring carries the weight stream). First 512 columns land
            # first so the first PSUM group can start early.
            c0 = min(512, cap_m)
            nc.scalar.dma_start(xm_s[:, :, 0:c0], xm_d[:, :, 0:c0])
            if c0 < cap_m:
                nc.scalar.dma_start(xm_s[:, :, c0:], xm_d[:, :, c0:])
            if cap_s:
                nc.scalar.dma_start(xs_s[:], xs_d[:])

            # ---- Phase 1: hT = silu(w1 @ xT) * (w3 @ xT), SBUF-resident ----
            with (
                tc.tile_pool(name="w13", bufs=3) as wpool,
                tc.tile_pool(name="act", bufs=3) as spool,
                tc.tile_pool(name="ps1", bufs=3, space="PSUM") as psum1,
            ):
                def swiglu_block(w1c, w3c, x_s, h_s, f, t0, tn):
                    h1p = psum1.tile([128, tn], F32, tag="h1p")
                    h3p = psum1.tile([128, tn], F32, tag="h3p")
                    for k in range(KD):
                        nc.tensor.matmul(
                            h1p[:], w1c[:, k, :], x_s[:, k, t0:t0 + tn],
                            start=(k == 0), stop=(k == KD - 1))
                    for k in range(KD):
                        nc.tensor.matmul(
                            h3p[:], w3c[:, k, :], x_s[:, k, t0:t0 + tn],
                            start=(k == 0), stop=(k == KD - 1))
                    s1 = spool.tile([128, tn], F32, tag="s1")
                    nc.scalar.activation(s1[:], h1p[:], silu)
                    nc.vector.tensor_mul(h_s[:, f, t0:t0 + tn], s1[:], h3p[:])

                for f in range(KF):
                    w1mc = wpool.tile([128, KD, 128], F16, tag="w1m")
                    nc.sync.dma_start(w1mc[:], w1m_d[f])
                    w3mc = wpool.tile([128, KD, 128], F16, tag="w3m")
                    nc.sync.dma_start(w3mc[:], w3m_d[f])
                    if cap_s:
                        w1sc = wpool.tile([128, KD, 128], F16, tag="w1s")
                        nc.sync.dma_start(w1sc[:], w1s_d[f])
                        w3sc = wpool.tile([128, KD, 128], F16, tag="w3s")
                        nc.sync.dma_start(w3sc[:], w3s_d[f])
                    for (t0, tn) in tiles_m:
                        swiglu_block(w1mc, w3mc, xm_s, hm_s, f, t0, tn)
                    if cap_s:
                        swiglu_block(w1sc, w3sc, xs_s, hs_s, f, 0, cap_s)

            # ---- Phase 2: yT = w2 @ hT ----
            with (
                tc.tile_pool(name="w2", bufs=3) as w2pool,
                tc.tile_pool(name="yo", bufs=3) as ypool,
                tc.tile_pool(name="ps2", bufs=4, space="PSUM") as psum2,
            ):
                def out_block(w2c, h_s, y_d, m, t0, tn):
                    yp = psum2.tile([128, tn], F32, tag="yp")
                    for k2 in range(KF):
                        nc.tensor.matmul(
                            yp[:], w2c[:, k2, :], h_s[:, k2, t0:t0 + tn],
                            start=(k2 == 0), stop=(k2 == KF - 1))
                    yo = ypool.tile([128, tn], F16, tag="yo")
                    nc.vector.tensor_copy(yo[:], yp[:])
                    nc.scalar.dma_start(y_d[m][:, t0:t0 + tn], yo[:])

                for m in range(KD):
                    w2mc = w2pool.tile([128, KF, 128], F16, tag="w2m")
                    nc.sync.dma_start(w2mc[:], w2m_d[m])
                    if cap_s:
                        w2sc = w2pool.tile([128, KF, 128], F16, tag="w2s")
                        nc.sync.dma_start(w2sc[:], w2s_d[m])
                    for (t0, tn) in tiles_m:
                        out_block(w2mc, hm_s, ym_d, m, t0, tn)
                    if cap_s:
                        out_block(w2sc, hs_s, ys_d, m, 0, cap_s)

    nc.compile()
    return nc


def _plan(counts):
    """Pick (cap_m, cap_s): every core computes cap_m tokens of its own
    expert plus one cap_s-token side block of an overflowing expert.
    Minimizes cap_m + cap_s subject to total overflow chunks <= N_CORES."""
    mx = int(max(counts))
    cap0 = max(512, -(-mx // 16) * 16)      # no-side fallback
    best = (cap0, cap0, 0)                  # (cost, cap_m, cap_s)
    for s in (32, 40, 48, 56, 64, 72, 80, 96, 112, 128):
        lo = max(512, mx - s * N_CORES)
        for cap_m in range(-(-lo // 4) * 4, mx + 1, 4):
            need = sum(-(-max(0, int(n) - cap_m) // s) for n in counts)
            if need <= N_CORES:
                cost = cap_m + s
                if cost < best[0] or (cost == best[0]
                                      and abs(s - 64) < abs(best[2] - 64)):
                    best = (cost, cap_m, s)
                break
    _, cap_m, cap_s = best
    return (cap_m, cap_s) if cap_s and cap_m + cap_s < cap0 else (cap0, 0)


def kernel(x, expert_indices, expert_weights, w1, w2, w3):
    global LAST_EXEC_NS
    x = np.ascontiguousarray(np.asarray(x, dtype=np.float32))
    routing = np.asarray(expert_indices)
    probs = np.asarray(expert_weights, dtype=np.float32)
    w1 = np.asarray(w1, dtype=np.float32)
    w2 = np.asarray(w2, dtype=np.float32)
    w3 = np.asarray(w3, dtype=np.float32)
    n_tokens = x.shape[0]

    idxs = [np.flatnonzero(routing[:, e]) for e in range(NUM_EXPERTS)]
    counts = [len(i) for i in idxs]
    cap_m, cap_s = _plan(counts)

    # Assign overflow chunks (expert, start, count) to the 8 side slots
    slots = []
    if cap_s:
        for e in range(NUM_EXPERTS):
            off = cap_m
            while off < counts[e]:
                cnt = min(cap_s, counts[e] - off)
                slots.append((e, off, cnt))
                off += cnt
        assert len(slots) <= N_CORES, (cap_m, cap_s, counts)
    slots += [None] * (N_CORES - len(slots))

    if (cap_m, cap_s) not in _PROGRAMS:
        _PROGRAMS[(cap_m, cap_s)] = _build_program(cap_m, cap_s)
    nc = _PROGRAMS[(cap_m, cap_s)]

    x16 = x.astype(np.float16)
    w1_16 = w1.astype(np.float16)
    w3_16 = w3.astype(np.float16)
    w2_16 = w2.astype(np.float16)

    def _wprep(e):
        # W1T[f,p,k,m] = w1[e][f*128+m, k*128+p]; W2T[m,p,k2,d] = w2[e][m*128+d, k2*128+p]
        return (
            np.ascontiguousarray(
                w1_16[e].reshape(KF, 128, KD, 128).transpose(0, 3, 2, 1)),
            np.ascontiguousarray(
                w3_16[e].reshape(KF, 128, KD, 128).transpose(0, 3, 2, 1)),
            np.ascontiguousarray(
                w2_16[e].reshape(KD, 128, KF, 128).transpose(0, 3, 2, 1)),
        )

    from concurrent.futures import ThreadPoolExecutor
    with ThreadPoolExecutor(max_workers=NUM_EXPERTS) as pool:
        wt = list(pool.map(_wprep, range(NUM_EXPERTS)))

    def _xgather(idx, cap):
        # [128, KD, cap] partition-major: out[p, k, t] = x[idx[t], k*128+p]
        out = np.zeros((128, KD, cap), dtype=np.float16)
        if len(idx):
            out[:, :, :len(idx)] = (
                x16[idx].T.reshape(KD, 128, len(idx)).transpose(1, 0, 2))
        return out

    zero_w = None
    in_maps = []
    for c in range(N_CORES):
        m = {
            "xm": _xgather(idxs[c][:cap_m], cap_m),
            "w1m": wt[c][0], "w3m": wt[c][1], "w2m": wt[c][2],
        }
        if cap_s:
            if slots[c] is not None:
                e, off, cnt = slots[c]
                m["xs"] = _xgather(idxs[e][off:off + cnt], cap_s)
                m["w1s"], m["w3s"], m["w2s"] = wt[e]
            else:
                if zero_w is None:
                    zero_w = (
                        np.zeros((128, KD, cap_s), np.float16),
                        np.zeros((KF, 128, KD, 128), np.float16),
                        np.zeros((KD, 128, KF, 128), np.float16),
                    )
                m["xs"] = zero_w[0]
                m["w1s"] = m["w3s"] = zero_w[1]
                m["w2s"] = zero_w[2]
        in_maps.append(m)

    trace = os.environ.get("BASS_KERNEL_TRACE", "0") == "1"
    if trace:
        import importlib.util
        if importlib.util.find_spec("antenv") is None or importlib.util.find_spec(
                "antenv.axon_hooks") is None:
            trace = False  # NTFF hook unavailable in this environment
    res = run_bass_kernel_spmd(
        nc, in_maps, core_ids=list(range(N_CORES)),
        trace=trace, trace_cores=list(range(N_CORES)) if trace else None,
    )
    LAST_EXEC_NS = res.exec_time_ns

    out = np.zeros((n_tokens, DIM), dtype=np.float32)
    for e in range(NUM_EXPERTS):
        idx = idxs[e][:cap_m]
        y_t = res.results[e]["ym"].reshape(DIM, cap_m)[:, :len(idx)]
        out[idx] += probs[idx, e][:, None] * y_t.T.astype(np.float32)
    for c in range(N_CORES):
        if cap_s and slots[c] is not None:
            e, off, cnt = slots[c]
            idx = idxs[e][off:off + cnt]
            y_t = res.results[c]["ys"].reshape(DIM, cap_s)[:, :cnt]
            out[idx] += probs[idx, e][:, None] * y_t.T.astype(np.float32)
    return out


# revision 33
# speedup vs baseline: 1.1856x; 1.0195x over previous
"""Trainium2 Bass kernel for nn_ConditionalFeedForward (MoE top-2 routing).

Strategy: expert-parallel across 8 NeuronCores with a load-balancing "side
block". Core e owns expert e's weights and computes the first CAP_M routed
tokens of expert e; the overflow tokens of heavy experts are distributed as
<=CAP_S-token side blocks to other cores (each core carries one side block
with its own small weight stream). All matmul operands are fp16 (full
1 cycle/row PE rate, half the HBM traffic of fp32r); PSUM accumulates fp32.

Single fused pass per core, hT kept resident in SBUF (no DRAM staging):

    hT = silu(w1 @ xT) * (w3 @ xT)     # [FFN, CAP] fp16 slab in SBUF
    yT = w2 @ hT                       # [DIM, CAP] -> fp16 out

Host gathers/pads tokens per expert, pre-transposes weights into PE layouts,
and scatter-adds gate-weighted outputs back to the full [N_TOKENS, DIM]
result in fp32.
"""

import os
import numpy as np

import concourse.bacc as bacc
import concourse.mybir as mybir
import concourse.tile as tile
from concourse.bass_utils import run_bass_kernel_spmd

# Problem constants (hardcoded per harness contract)
NUM_EXPERTS = 8
DIM = 2048
FFN = 5632
N_CORES = 8
KD = DIM // 128    # 16 contraction chunks for GEMM1/3; output chunks GEMM2
KF = FFN // 128    # 44 ffn chunks

F32 = mybir.dt.float32
F16 = mybir.dt.float16

# Compiled program cache keyed by (cap_m, cap_s)
_PROGRAMS = {}

# Filled by the last kernel() call when BASS_KERNEL_TRACE=1 (for test.py)
LAST_EXEC_NS = None


def _tiles(total, mx=512):
    """Token tiles of <=512 (PSUM bank = 512 fp32)."""
    return [(t0, min(mx, total - t0)) for t0 in range(0, total, mx)]


def _xblocks(cap):
    """Column blocks for the x load: small leading blocks so the first PSUM
    groups start early, each a contiguous DRAM tensor (128 descriptors)."""
    if cap > 512:
        return [(0, 256), (256, 256)] + [(512 + t0, tn)
                                         for t0, tn in _tiles(cap - 512)]
    return _tiles(cap, 256)


def _build_program(cap_m, cap_s):
    nc = bacc.Bacc("TRN2", target_bir_lowering=False, debug=False,
                   num_devices=N_CORES)

    xblocks = _xblocks(cap_m)
    xm_d = [nc.dram_tensor(f"xm{j}", [128, KD, bw], F16, kind="ExternalInput")
            for j, (b0, bw) in enumerate(xblocks)]
    w1m_d = nc.dram_tensor("w1m", [KF, 128, KD, 128], F16, kind="ExternalInput")
    w3m_d = nc.dram_tensor("w3m", [KF, 128, KD, 128], F16, kind="ExternalInput")
    w2m_d = nc.dram_tensor("w2m", [KD, 128, KF, 128], F16, kind="ExternalInput")
    ym_d = nc.dram_tensor("ym", [KD, 128, cap_m], F16, kind="ExternalOutput")
    if cap_s:
        xs_d = nc.dram_tensor("xs", [128, KD, cap_s], F16, kind="ExternalInput")
        w1s_d = nc.dram_tensor("w1s", [KF, 128, KD, 128], F16, kind="ExternalInput")
        w3s_d = nc.dram_tensor("w3s", [KF, 128, KD, 128], F16, kind="ExternalInput")
        w2s_d = nc.dram_tensor("w2s", [KD, 128, KF, 128], F16, kind="ExternalInput")
        ys_d = nc.dram_tensor("ys", [KD, 128, cap_s], F16, kind="ExternalOutput")

    silu = mybir.ActivationFunctionType.Silu
    tiles_m = _tiles(cap_m)

    with tile.TileContext(nc) as tc:
        with (
            tc.tile_pool(name="x", bufs=1) as xpool,
            tc.tile_pool(name="h", bufs=1) as hpool,
            tc.tile_pool(name="w2h", bufs=1) as w2hpool,
        ):
            xb_s = [xpool.tile([128, KD, bw], F16, name=f"xb{j}")
                    for j, (b0, bw) in enumerate(xblocks)]
            hm_s = hpool.tile([128, KF, cap_m], F16)
            if cap_s:
                xs_s = xpool.tile([128, KD, cap_s], F16)
                hs_s = hpool.tile([128, KF, cap_s], F16)
            # x loads on the ACT HWDGE ring (contiguous per-block DMAs; the
            # SP ring carries the weight stream), first-needed first: the
            # side block computes first, so its tiny x lands first. Blocks
            # >=2 are issued inside the f-loop so the early f's weight
            # chunks aren't queued behind them on the shared DMA engines.
            if cap_s:
                nc.scalar.dma_start(xs_s[:], xs_d[:])
            for j in range(min(2, len(xblocks))):
                nc.scalar.dma_start(xb_s[j][:], xm_d[j][:])
            w2m0 = w2hpool.tile([128, KF, 128], F16)
            if cap_s:
                w2s0 = w2hpool.tile([128, KF, 128], F16)

            # ---- Phase 1: hT = silu(w1 @ xT) * (w3 @ xT), SBUF-resident ----
            with (
                tc.tile_pool(name="w13", bufs=3) as wpool,
                tc.tile_pool(name="act", bufs=3) as spool,
                tc.tile_pool(name="ps1", bufs=3, space="PSUM") as psum1,
            ):
                def swiglu_block(w1c, w3c, x_t, h_s, f, g0, tn):
                    # x_t: per-block x tile (read at local offset 0);
                    # g0: global token offset for the h-slab write
                    h1p = psum1.tile([128, tn], F32, tag="h1p")
                    h3p = psum1.tile([128, tn], F32, tag="h3p")
                    for k in range(KD):
                        nc.tensor.matmul(
                            h1p[:], w1c[:, k, :], x_t[:, k, 0:tn],
                            start=(k == 0), stop=(k == KD - 1))
                    for k in range(KD):
                        nc.tensor.matmul(
                            h3p[:], w3c[:, k, :], x_t[:, k, 0:tn],
                            start=(k == 0), stop=(k == KD - 1))
                    s1 = spool.tile([128, tn], F32, tag="s1")
                    nc.scalar.activation(s1[:], h1p[:], silu)
                    nc.vector.tensor_mul(h_s[:, f, g0:g0 + tn], s1[:], h3p[:])

                for f in range(KF):
                    # side first everywhere: its x/weights are first in the
                    # DMA queues, and at f=KF-1 its hs slab completes early
                    # so GEMM2 starts with zero boundary gap.
                    # w1 streams on the SP ring, w3 on the gpsimd SWDGE —
                    # two rings so issue overhead doesn't serialize.
                    if cap_s:
                        w1sc = wpool.tile([128, KD, 128], F16, tag="w1s")
                        nc.sync.dma_start(w1sc[:], w1s_d[f])
                        w3sc = wpool.tile([128, KD, 128], F16, tag="w3s")
                        nc.gpsimd.dma_start(w3sc[:], w3s_d[f])
                    w1mc = wpool.tile([128, KD, 128], F16, tag="w1m")
                    nc.sync.dma_start(w1mc[:], w1m_d[f])
                    w3mc = wpool.tile([128, KD, 128], F16, tag="w3m")
                    nc.gpsimd.dma_start(w3mc[:], w3m_d[f])
                    if f == KF - 2:
                        # m=0 GEMM2 weights prefetch on the now-idle ACT
                        # ring, with DMA-bus slack (startup is long past)
                        nc.scalar.dma_start(w2m0[:], w2m_d[0])
                        if cap_s:
                            nc.scalar.dma_start(w2s0[:], w2s_d[0])

                    if cap_s:
                        swiglu_block(w1sc, w3sc, xs_s, hs_s, f, 0, cap_s)
                    for j in range(len(xblocks)):
                        if f == 0 and j >= 2:
                            # deferred issue just before first use: keeps
                            # these behind the early weight chunks in the
                            # DMA queue (ACT stream throttles naturally)
                            nc.scalar.dma_start(xb_s[j][:], xm_d[j][:])
                        b0, bw = xblocks[j]
                        swiglu_block(w1mc, w3mc, xb_s[j], hm_s, f, b0, bw)

            # ---- Phase 2: yT = w2 @ hT ----
            with (
                tc.tile_pool(name="w2", bufs=2) as w2pool,
                tc.tile_pool(name="yo", bufs=3) as ypool,
                tc.tile_pool(name="ps2", bufs=4, space="PSUM") as psum2,
            ):
                def out_block(w2c, h_s, y_d, m, t0, tn):
                    yp = psum2.tile([128, tn], F32, tag="yp")
                    for k2 in range(KF):
                        nc.tensor.matmul(
                            yp[:], w2c[:, k2, :], h_s[:, k2, t0:t0 + tn],
                            start=(k2 == 0), stop=(k2 == KF - 1))
                    yo = ypool.tile([128, tn], F16, tag="yo")
                    nc.vector.tensor_copy(yo[:], yp[:])
                    nc.scalar.dma_start(y_d[m][:, t0:t0 + tn], yo[:])

                for m in range(KD):
                    if m == 0:
                        w2mc, w2sc = w2m0, (w2s0 if cap_s else None)
                    else:
                        w2mc = w2pool.tile([128, KF, 128], F16, tag="w2m")
                        nc.sync.dma_start(w2mc[:], w2m_d[m])
                        if cap_s:
                            w2sc = w2pool.tile([128, KF, 128], F16, tag="w2s")
                            nc.sync.dma_start(w2sc[:], w2s_d[m])
                    # m=0: side first (its hs slab is complete earliest, so
                    # GEMM2 starts with no boundary gap); else side last so
                    # the final drain is the tiny side tile.
                    if cap_s and m == 0:
                        out_block(w2sc, hs_s, ys_d, m, 0, cap_s)
                    for (t0, tn) in tiles_m:
                        out_block(w2mc, hm_s, ym_d, m, t0, tn)
                    if cap_s and m > 0:
                        out_block(w2sc, hs_s, ys_d, m, 0, cap_s)

    nc.compile()
    return nc


def _plan(counts):
    """Pick (cap_m, cap_s): every core computes cap_m tokens of its own
    expert plus one cap_s-token side block of an overflowing expert.
    Minimizes cap_m + cap_s subject to total overflow chunks <= N_CORES."""
    mx = int(max(counts))
    cap0 = max(512, -(-mx // 16) * 16)      # no-side fallback
    best = (cap0, cap0, 0)                  # (cost, cap_m, cap_s)
    for s in (32, 40, 48, 56, 64, 72, 80, 96, 112, 128):
        lo = max(512, mx - s * N_CORES)
        for cap_m in range(-(-lo // 4) * 4, mx + 1, 4):
            need = sum(-(-max(0, int(n) - cap_m) // s) for n in counts)
            if need <= N_CORES:
                cost = cap_m + s
                if cost < best[0] or (cost == best[0]
                                      and abs(s - 64) < abs(best[2] - 64)):
                    best = (cost, cap_m, s)
                break
    _, cap_m, cap_s = best
    return (cap_m, cap_s) if cap_s and cap_m + cap_s < cap0 else (cap0, 0)


def kernel(x, expert_indices, expert_weights, w1, w2, w3):
    global LAST_EXEC_NS
    x = np.ascontiguousarray(np.asarray(x, dtype=np.float32))
    routing = np.asarray(expert_indices)
    probs = np.asarray(expert_weights, dtype=np.float32)
    w1 = np.asarray(w1, dtype=np.float32)
    w2 = np.asarray(w2, dtype=np.float32)
    w3 = np.asarray(w3, dtype=np.float32)
    n_tokens = x.shape[0]

    idxs = [np.flatnonzero(routing[:, e]) for e in range(NUM_EXPERTS)]
    counts = [len(i) for i in idxs]
    cap_m, cap_s = _plan(counts)

    # Assign overflow chunks (expert, start, count) to the 8 side slots
    slots = []
    if cap_s:
        for e in range(NUM_EXPERTS):
            off = cap_m
            while off < counts[e]:
                cnt = min(cap_s, counts[e] - off)
                slots.append((e, off, cnt))
                off += cnt
        assert len(slots) <= N_CORES, (cap_m, cap_s, counts)
    slots += [None] * (N_CORES - len(slots))

    if (cap_m, cap_s) not in _PROGRAMS:
        _PROGRAMS[(cap_m, cap_s)] = _build_program(cap_m, cap_s)
    nc = _PROGRAMS[(cap_m, cap_s)]

    x16 = x.astype(np.float16)
    w1_16 = w1.astype(np.float16)
    w3_16 = w3.astype(np.float16)
    w2_16 = w2.astype(np.float16)

    def _wprep(e):
        # W1T[f,p,k,m] = w1[e][f*128+m, k*128+p]; W2T[m,p,k2,d] = w2[e][m*128+d, k2*128+p]
        return (
            np.ascontiguousarray(
                w1_16[e].reshape(KF, 128, KD, 128).transpose(0, 3, 2, 1)),
            np.ascontiguousarray(
                w3_16[e].reshape(KF, 128, KD, 128).transpose(0, 3, 2, 1)),
            np.ascontiguousarray(
                w2_16[e].reshape(KD, 128, KF, 128).transpose(0, 3, 2, 1)),
        )

    from concurrent.futures import ThreadPoolExecutor
    with ThreadPoolExecutor(max_workers=NUM_EXPERTS) as pool:
        wt = list(pool.map(_wprep, range(NUM_EXPERTS)))

    def _xgather(idx, cap):
        # [128, KD, cap] partition-major: out[p, k, t] = x[idx[t], k*128+p]
        out = np.zeros((128, KD, cap), dtype=np.float16)
        if len(idx):
            out[:, :, :len(idx)] = (
                x16[idx].T.reshape(KD, 128, len(idx)).transpose(1, 0, 2))
        return out

    xblocks = _xblocks(cap_m)
    zero_w = None
    in_maps = []
    for c in range(N_CORES):
        xmh = _xgather(idxs[c][:cap_m], cap_m)
        m = {"w1m": wt[c][0], "w3m": wt[c][1], "w2m": wt[c][2]}
        for j, (b0, bw) in enumerate(xblocks):
            m[f"xm{j}"] = np.ascontiguousarray(xmh[:, :, b0:b0 + bw])
        if cap_s:
            if slots[c] is not None:
                e, off, cnt = slots[c]
                m["xs"] = _xgather(idxs[e][off:off + cnt], cap_s)
                m["w1s"], m["w3s"], m["w2s"] = wt[e]
            else:
                if zero_w is None:
                    zero_w = (
                        np.zeros((128, KD, cap_s), np.float16),
                        np.zeros((KF, 128, KD, 128), np.float16),
                        np.zeros((KD, 128, KF, 128), np.float16),
                    )
                m["xs"] = zero_w[0]
                m["w1s"] = m["w3s"] = zero_w[1]
                m["w2s"] = zero_w[2]
        in_maps.append(m)

    trace = os.environ.get("BASS_KERNEL_TRACE", "0") == "1"
    if trace:
        import importlib.util
        if importlib.util.find_spec("antenv") is None or importlib.util.find_spec(
                "antenv.axon_hooks") is None:
            trace = False  # NTFF hook unavailable in this environment
    res = run_bass_kernel_spmd(
        nc, in_maps, core_ids=list(range(N_CORES)),
        trace=trace, trace_cores=list(range(N_CORES)) if trace else None,
    )
    LAST_EXEC_NS = res.exec_time_ns

    out = np.zeros((n_tokens, DIM), dtype=np.float32)
    for e in range(NUM_EXPERTS):
        idx = idxs[e][:cap_m]
        y_t = res.results[e]["ym"].reshape(DIM, cap_m)[:, :len(idx)]
        out[idx] += probs[idx, e][:, None] * y_t.T.astype(np.float32)
    for c in range(N_CORES):
        if cap_s and slots[c] is not None:
            e, off, cnt = slots[c]
            idx = idxs[e][off:off + cnt]
            y_t = res.results[c]["ys"].reshape(DIM, cap_s)[:, :cnt]
            out[idx] += probs[idx, e][:, None] * y_t.T.astype(np.float32)
    return out
